# revision 8
# baseline (speedup 1.0000x reference)
"""Trainium2 Bass kernel for nn_CustomLoss (CrossEntropy + binary-remap BCE).

loss = mean_i[ logsumexp(pred_i) - pred_i[t_i] ]
     + 100 * mean_i[ 1{ LUT[argmax(pred_i)] != LUT[t_i] } ]

with LUT = [0,0,1,1,1,1,1,1,0,0]  (LUT[j] = 1 iff 2 <= j <= 7).

Data-parallel over the batch across 8 NeuronCores.  The host re-packs each
row so the device needs neither a gather nor a sign trick:

  * rows are partitioned into region A (binary target bt=0, t in {0,1,8,9})
    and region B (bt=1, t in 2..7);
  * each row's 10 logits are permuted to  [X-group | pred[t] | Y-group-rest]
    where X = the class group NOT containing t and Y = the group containing
    t.  Then for every row, mismatch == (max X > max Y), and pred[t] sits at
    a fixed slot (6 in region A, 4 in region B);
  * everything ships as bf16 (22B/row less DMA than f32), +16 pre-shift so
    exp(x-16) = exp(pred).

Device per tile (region-typed widths, compile-time slot bounds):
  ACT   : E = exp(row - 16) (one flat bf16 instr), Ln(rowsum) with
          per-partition accumulate.  Single Exp+Ln table (no reloads).
  DVE   : packed-bf16 tensor_tensor add/max trees (2-byte packed operands
          hit the 2x/4x DVE fast modes), strided slice reduce for sum of
          pred[t], and one scalar_tensor_tensor is_gt with accum_out for
          the mismatch count.
  GPSIMD: the two strided tree tails (cheap, keeps DVE free).
"""

import numpy as np
import ml_dtypes

# ---------------------------------------------------------------- constants
N = 2_000_000
C = 10
N_CORES = 8
P = 128
ROWS_CORE = N // N_CORES      # 250,000
SHIFT = 0.0
MID = (2, 3, 4, 5, 6, 7)      # classes with LUT == 1
OUTER = (0, 1, 8, 9)          # classes with LUT == 0

# slot permutation per target class: [X-group | t | Y-group minus t]
_IDX_TAB = np.empty((C, C), np.int64)
for _t in range(C):
    if _t in OUTER:   # region A: X = mid(6), Y = t + outer\{t}(3)
        _IDX_TAB[_t] = list(MID) + [_t] + [c for c in OUTER if c != _t]
    else:             # region B: X = outer(4), Y = t + mid\{t}(5)
        _IDX_TAB[_t] = list(OUTER) + [_t] + [c for c in MID if c != _t]

_CACHE = {}


def _split_w(w, first=110, body=352):
    """Tile widths: small first tile to prime the pipeline."""
    ws = []
    if w > first + 64:
        ws.append(first)
        w -= first
    while w > 0:
        c = min(body, w)
        if 0 < w - c < 64:
            c = w          # avoid a tiny trailing tile
        ws.append(c)
        w -= c
    return ws


# ------------------------------------------------------------- device build
def _build_nc(tiles_a, tiles_b):
    import concourse.bass as bass  # noqa: F401  (env setup)
    import concourse.tile as tile
    from concourse import bacc, mybir

    f32 = mybir.dt.float32
    bf16 = mybir.dt.bfloat16
    A = mybir.ActivationFunctionType
    X = mybir.AxisListType.X
    alu = mybir.AluOpType

    tiles = [("a", w) for w in tiles_a] + [("b", w) for w in tiles_b]
    T = len(tiles)

    nc = bacc.Bacc("TRN2", target_bir_lowering=False, debug=False,
                   num_devices=N_CORES)
    comb_ds = [
        nc.dram_tensor(f"comb{i}", [P, w * C], bf16, kind="ExternalInput").ap()
        for i, (_, w) in enumerate(tiles)
    ]
    out_d = nc.dram_tensor("out", [P, 4], f32, kind="ExternalOutput").ap()

    with tile.TileContext(nc) as tc:
        with (
            tc.tile_pool(name="io", bufs=3) as io,
            tc.tile_pool(name="ep", bufs=3) as ep,
            tc.tile_pool(name="zp", bufs=3) as zp,
            tc.tile_pool(name="mp", bufs=3) as mp,
            tc.tile_pool(name="cp", bufs=1) as cp,
        ):
            bias16 = cp.tile([P, 1], f32)
            nc.gpsimd.memset(bias16[:], -SHIFT)
            acc_all = cp.tile([P, 4, T], f32)
            acc_lg = acc_all[:, 0, :]
            acc_g = acc_all[:, 1, :]
            acc_mm = acc_all[:, 2, :]
            acc_me = acc_all[:, 3, :]

            for i, (reg, w) in enumerate(tiles):
                ct = io.tile([P, w * C], bf16, tag="comb")
                nc.sync.dma_start(ct[:], comb_ds[i])
                cv = ct[:].rearrange("p (w s) -> p w s", s=C)

                # ---- CE: exp (flat), packed add tree, ln with accumulate
                et = ep.tile([P, w * C], bf16, tag="E")
                nc.scalar.activation(et[:], ct[:], A.Exp, bias=bias16[:])
                ev = et[:].rearrange("p (w s) -> p w s", s=C)

                z = zp.tile([P, w, 5], bf16, tag="z")
                nc.vector.tensor_tensor(z[:], ev[:, :, 0:5], ev[:, :, 5:10],
                                        op=alu.add)
                cp2 = zp.tile([P, w, 2], bf16, tag="c")
                nc.vector.tensor_tensor(cp2[:], z[:, :, 0:2], z[:, :, 2:4],
                                        op=alu.add)
                d = zp.tile([P, w], bf16, tag="d")
                nc.gpsimd.tensor_tensor(d[:], cp2[:, :, 0], cp2[:, :, 1],
                                        op=alu.add)
                s = zp.tile([P, w], bf16, tag="s")
                nc.gpsimd.tensor_tensor(s[:], d[:], z[:, :, 4], op=alu.add)
                lg = zp.tile([P, w], f32, tag="lg")
                nc.scalar.activation(lg[:], s[:], A.Ln,
                                     accum_out=acc_lg[:, i:i + 1])

                # ---- sum of pred'[t]: fixed slot per region
                u = 6 if reg == "a" else 4
                nc.vector.reduce_sum(acc_g[:, i:i + 1], cv[:, :, u], axis=X)

                # ---- BCE: packed max trees, then one is_gt with accum
                if reg == "a":
                    m1 = mp.tile([P, w, 2], bf16, tag="m1")
                    nc.vector.tensor_tensor(m1[:], cv[:, :, 0:2],
                                            cv[:, :, 2:4], op=alu.max)
                    m2 = mp.tile([P, w, 2], bf16, tag="m2")
                    nc.vector.tensor_tensor(m2[:], m1[:], cv[:, :, 4:6],
                                            op=alu.max)
                    mx = mp.tile([P, w], bf16, tag="mx")
                    nc.vector.tensor_tensor(mx[:], m2[:, :, 0], m2[:, :, 1],
                                            op=alu.max)
                    n1 = mp.tile([P, w, 2], bf16, tag="n1")
                    nc.vector.tensor_tensor(n1[:], cv[:, :, 6:8],
                                            cv[:, :, 8:10], op=alu.max)
                    my = mp.tile([P, w], bf16, tag="my")
                    nc.vector.tensor_tensor(my[:], n1[:, :, 0], n1[:, :, 1],
                                            op=alu.max)
                else:
                    m1 = mp.tile([P, w, 2], bf16, tag="m1")
                    nc.vector.tensor_tensor(m1[:], cv[:, :, 0:2],
                                            cv[:, :, 2:4], op=alu.max)
                    mx = mp.tile([P, w], bf16, tag="mx")
                    nc.vector.tensor_tensor(mx[:], m1[:, :, 0], m1[:, :, 1],
                                            op=alu.max)
                    n1 = mp.tile([P, w, 2], bf16, tag="n1")
                    nc.vector.tensor_tensor(n1[:], cv[:, :, 4:6],
                                            cv[:, :, 6:8], op=alu.max)
                    n2 = mp.tile([P, w, 2], bf16, tag="n2")
                    nc.vector.tensor_tensor(n2[:], n1[:], cv[:, :, 8:10],
                                            op=alu.max)
                    my = mp.tile([P, w], bf16, tag="my")
                    nc.vector.tensor_tensor(my[:], n2[:, :, 0], n2[:, :, 1],
                                            op=alu.max)
                # bf16 rounding is monotone, so mx/my are exactly the rounded
                # group maxes; ties (equal after rounding) are true-greater
                # with probability ~1/2.  Count (is_gt + is_ge)/2.
                q = mp.tile([P, w], bf16, tag="q")
                nc.vector.scalar_tensor_tensor(
                    q[:], mx[:], 0.0, my[:], op0=alu.add, op1=alu.is_gt,
                    accum_out=acc_mm[:, i:i + 1])
                q2 = mp.tile([P, w], bf16, tag="q2")
                nc.vector.scalar_tensor_tensor(
                    q2[:], mx[:], 0.0, my[:], op0=alu.add, op1=alu.is_ge,
                    accum_out=acc_me[:, i:i + 1])

            # ---- final per-partition fold + store
            out_t = cp.tile([P, 4], f32)
            nc.vector.reduce_sum(out_t[:], acc_all[:], axis=X)
            nc.sync.dma_start(out_d[:], out_t[:])

    # Single activation table containing both Exp and Ln so the compiler
    # does not ping-pong ACT_TABLE_LOADs.
    import concourse.bacc as bacc_mod
    from concourse.hw_specs import get_activation_tables
    orig = get_activation_tables(nc.m.arch)
    combined = None
    for k, v in orig.items():
        if (mybir.ActivationFunctionType.Exp in v
                and mybir.ActivationFunctionType.Ln in v):
            combined = k
            break
    if combined is not None:
        patched = {k: (v if k == combined else set()) for k, v in orig.items()}
        saved = bacc_mod.get_activation_tables
        bacc_mod.get_activation_tables = lambda arch: patched
        try:
            nc.compile()
        finally:
            bacc_mod.get_activation_tables = saved
    else:
        nc.compile()
    return nc


def _get_nc():
    key = _CACHE["tiles_key"]
    if ("nc", key) not in _CACHE:
        _CACHE[("nc", key)] = _build_nc(*key)
    return _CACHE[("nc", key)]


# ------------------------------------------------------------------- host
def _host_prep(pred, target):
    """Shard, region-sort, permute slots, pad, tile. Returns in_maps and
    stores layout info in _CACHE."""
    pred = np.asarray(pred, dtype=np.float32)
    target = np.asarray(target).astype(np.int64)

    predp = pred.astype(ml_dtypes.bfloat16)
    colidx = _IDX_TAB[target]                      # [N, 10]
    packed = np.take_along_axis(predp, colidx, axis=1)
    bt = ((target >= 2) & (target <= 7))

    # per-core region rows
    core_a, core_b = [], []
    for c in range(N_CORES):
        sl = slice(c * ROWS_CORE, (c + 1) * ROWS_CORE)
        btc = bt[sl]
        pc = packed[sl]
        core_a.append(pc[~btc])
        core_b.append(pc[btc])

    wa = max((a.shape[0] + P - 1) // P for a in core_a)
    wb = max((b.shape[0] + P - 1) // P for b in core_b)
    tiles_a = _split_w(wa)
    tiles_b = _split_w(wb)
    _CACHE["tiles_key"] = (tuple(tiles_a), tuple(tiles_b))

    n_pad = 0
    in_maps = []
    for c in range(N_CORES):
        m = {}
        i = 0
        for rows_all, w_tot, tiles in ((core_a[c], wa, tiles_a),
                                       (core_b[c], wb, tiles_b)):
            n = rows_all.shape[0]
            pad = P * w_tot - n
            n_pad += pad
            if pad:
                rows_all = np.concatenate(
                    [rows_all, np.zeros((pad, C), ml_dtypes.bfloat16)], axis=0)
            # [P*w, 10] -> [P, w, 10] partition-major
            rows_all = rows_all.reshape(P, w_tot, C)
            off = 0
            for w in tiles:
                m[f"comb{i}"] = np.ascontiguousarray(
                    rows_all[:, off:off + w, :].reshape(P, w * C))
                off += w
                i += 1
        in_maps.append(m)
    _CACHE["n_pad"] = n_pad
    return in_maps


def _pad_ln_const():
    """Pad rows are all-zero: E = exp(0) = 1 exactly, bf16 tree sums to 10
    exactly, so the device adds ln(10) per pad row."""
    return float(np.log(10.0))


def kernel(pred, target):
    from concourse.bass_utils import run_bass_kernel_spmd

    in_maps = _host_prep(pred, target)
    nc = _get_nc()
    res = run_bass_kernel_spmd(nc, in_maps, core_ids=list(range(N_CORES)))

    sum_lg = 0.0
    sum_g = 0.0
    sum_mm = 0.0
    for c in range(N_CORES):
        o = res.results[c]["out"].astype(np.float64)
        sum_lg += o[:, 0].sum()
        sum_g += o[:, 1].sum()
        sum_mm += 0.5 * (o[:, 2].sum() + o[:, 3].sum())

    # pad rows: lnS = pad const, pred[t] slot = 0, no mismatch
    sum_lg -= _CACHE["n_pad"] * _pad_ln_const()
    ce = (sum_lg - sum_g) / N
    bce = 100.0 * sum_mm / N
    return np.float32(ce + bce)


# revision 9
# speedup vs baseline: 1.0006x; 1.0006x over previous
"""Trainium2 Bass kernel for nn_CustomLoss (CrossEntropy + binary-remap BCE).

loss = mean_i[ logsumexp(pred_i) - pred_i[t_i] ]
     + 100 * mean_i[ 1{ LUT[argmax(pred_i)] != LUT[t_i] } ]

with LUT = [0,0,1,1,1,1,1,1,0,0]  (LUT[j] = 1 iff 2 <= j <= 7).

Data-parallel over the batch across 8 NeuronCores.  The host re-packs each
row so the device needs neither a gather nor a sign trick:

  * rows are partitioned into region A (binary target bt=0, t in {0,1,8,9})
    and region B (bt=1, t in 2..7);
  * each row's 10 logits are permuted to  [X-group | pred[t] | Y-group-rest]
    where X = the class group NOT containing t and Y = the group containing
    t.  Then for every row, mismatch == (max X > max Y), and pred[t] sits at
    a fixed slot (6 in region A, 4 in region B);
  * everything ships as bf16 (22B/row less DMA than f32), +16 pre-shift so
    exp(x-16) = exp(pred).

Device per tile (region-typed widths, compile-time slot bounds):
  ACT   : E = exp(row - 16) (one flat bf16 instr), Ln(rowsum) with
          per-partition accumulate.  Single Exp+Ln table (no reloads).
  DVE   : packed-bf16 tensor_tensor add/max trees (2-byte packed operands
          hit the 2x/4x DVE fast modes), strided slice reduce for sum of
          pred[t], and one scalar_tensor_tensor is_gt with accum_out for
          the mismatch count.
  GPSIMD: the two strided tree tails (cheap, keeps DVE free).
"""

import numpy as np
import ml_dtypes

# ---------------------------------------------------------------- constants
N = 2_000_000
C = 10
N_CORES = 8
P = 128
ROWS_CORE = N // N_CORES      # 250,000
SHIFT = 0.0
MID = (2, 3, 4, 5, 6, 7)      # classes with LUT == 1
OUTER = (0, 1, 8, 9)          # classes with LUT == 0

# slot permutation per target class: [X-group | t | Y-group minus t]
_IDX_TAB = np.empty((C, C), np.int64)
for _t in range(C):
    if _t in OUTER:   # region A: X = mid(6), Y = t + outer\{t}(3)
        _IDX_TAB[_t] = list(MID) + [_t] + [c for c in OUTER if c != _t]
    else:             # region B: X = outer(4), Y = t + mid\{t}(5)
        _IDX_TAB[_t] = list(OUTER) + [_t] + [c for c in MID if c != _t]

_CACHE = {}


def _split_w(w, first=110, body=352):
    """Tile widths: small first tile to prime the pipeline."""
    ws = []
    if w > first + 64:
        ws.append(first)
        w -= first
    while w > 0:
        c = min(body, w)
        if 0 < w - c < 64:
            c = w          # avoid a tiny trailing tile
        ws.append(c)
        w -= c
    return ws


# ------------------------------------------------------------- device build
def _build_nc(tiles_a, tiles_b):
    import concourse.bass as bass  # noqa: F401  (env setup)
    import concourse.tile as tile
    from concourse import bacc, mybir

    f32 = mybir.dt.float32
    bf16 = mybir.dt.bfloat16
    A = mybir.ActivationFunctionType
    X = mybir.AxisListType.X
    alu = mybir.AluOpType

    tiles = [("a", w) for w in tiles_a] + [("b", w) for w in tiles_b]
    T = len(tiles)

    nc = bacc.Bacc("TRN2", target_bir_lowering=False, debug=False,
                   num_devices=N_CORES)
    comb_ds = [
        nc.dram_tensor(f"comb{i}", [P, w * C], bf16, kind="ExternalInput").ap()
        for i, (_, w) in enumerate(tiles)
    ]
    out_d = nc.dram_tensor("out", [P, 4], f32, kind="ExternalOutput").ap()

    with tile.TileContext(nc) as tc:
        with (
            tc.tile_pool(name="io", bufs=3) as io,
            tc.tile_pool(name="ep", bufs=3) as ep,
            tc.tile_pool(name="zp", bufs=3) as zp,
            tc.tile_pool(name="mp", bufs=3) as mp,
            tc.tile_pool(name="cp", bufs=1) as cp,
        ):
            bias16 = cp.tile([P, 1], f32)
            nc.gpsimd.memset(bias16[:], -SHIFT)
            acc_all = cp.tile([P, 4, T], f32)
            acc_lg = acc_all[:, 0, :]
            acc_g = acc_all[:, 1, :]
            acc_mm = acc_all[:, 2, :]
            acc_me = acc_all[:, 3, :]

            for i, (reg, w) in enumerate(tiles):
                ct = io.tile([P, w * C], bf16, tag="comb")
                nc.sync.dma_start(ct[:], comb_ds[i])
                cv = ct[:].rearrange("p (w s) -> p w s", s=C)

                # ---- CE: exp (flat), packed add tree, ln with accumulate
                et = ep.tile([P, w * C], bf16, tag="E")
                nc.scalar.activation(et[:], ct[:], A.Exp, bias=bias16[:])
                ev = et[:].rearrange("p (w s) -> p w s", s=C)

                z = zp.tile([P, w, 5], bf16, tag="z")
                nc.vector.tensor_tensor(z[:], ev[:, :, 0:5], ev[:, :, 5:10],
                                        op=alu.add)
                cp2 = zp.tile([P, w, 2], bf16, tag="c")
                nc.vector.tensor_tensor(cp2[:], z[:, :, 0:2], z[:, :, 2:4],
                                        op=alu.add)
                d = zp.tile([P, w], bf16, tag="d")
                nc.gpsimd.tensor_tensor(d[:], cp2[:, :, 0], cp2[:, :, 1],
                                        op=alu.add)
                s = zp.tile([P, w], bf16, tag="s")
                nc.gpsimd.tensor_tensor(s[:], d[:], z[:, :, 4], op=alu.add)
                lg = zp.tile([P, w], f32, tag="lg")
                nc.scalar.activation(lg[:], s[:], A.Ln,
                                     accum_out=acc_lg[:, i:i + 1])

                # ---- sum of pred'[t]: fixed slot per region
                u = 6 if reg == "a" else 4
                nc.vector.reduce_sum(acc_g[:, i:i + 1], cv[:, :, u], axis=X)

                # ---- BCE: packed max trees, then one is_gt with accum
                if reg == "a":
                    m1 = mp.tile([P, w, 2], bf16, tag="m1")
                    nc.vector.tensor_tensor(m1[:], cv[:, :, 0:2],
                                            cv[:, :, 2:4], op=alu.max)
                    m2 = mp.tile([P, w, 2], bf16, tag="m2")
                    nc.vector.tensor_tensor(m2[:], m1[:], cv[:, :, 4:6],
                                            op=alu.max)
                    mx = mp.tile([P, w], bf16, tag="mx")
                    nc.vector.tensor_tensor(mx[:], m2[:, :, 0], m2[:, :, 1],
                                            op=alu.max)
                    n1 = mp.tile([P, w, 2], bf16, tag="n1")
                    nc.vector.tensor_tensor(n1[:], cv[:, :, 6:8],
                                            cv[:, :, 8:10], op=alu.max)
                    my = mp.tile([P, w], bf16, tag="my")
                    nc.vector.tensor_tensor(my[:], n1[:, :, 0], n1[:, :, 1],
                                            op=alu.max)
                else:
                    m1 = mp.tile([P, w, 2], bf16, tag="m1")
                    nc.vector.tensor_tensor(m1[:], cv[:, :, 0:2],
                                            cv[:, :, 2:4], op=alu.max)
                    mx = mp.tile([P, w], bf16, tag="mx")
                    nc.vector.tensor_tensor(mx[:], m1[:, :, 0], m1[:, :, 1],
                                            op=alu.max)
                    n1 = mp.tile([P, w, 2], bf16, tag="n1")
                    nc.vector.tensor_tensor(n1[:], cv[:, :, 4:6],
                                            cv[:, :, 6:8], op=alu.max)
                    n2 = mp.tile([P, w, 2], bf16, tag="n2")
                    nc.vector.tensor_tensor(n2[:], n1[:], cv[:, :, 8:10],
                                            op=alu.max)
                    my = mp.tile([P, w], bf16, tag="my")
                    nc.vector.tensor_tensor(my[:], n2[:, :, 0], n2[:, :, 1],
                                            op=alu.max)
                # bf16 rounding is monotone, so mx/my are exactly the rounded
                # group maxes; ties (equal after rounding) are true-greater
                # with probability ~1/2.  Count (is_gt + is_ge)/2.
                q = mp.tile([P, w], bf16, tag="q")
                nc.vector.scalar_tensor_tensor(
                    q[:], mx[:], 0.0, my[:], op0=alu.add, op1=alu.is_gt,
                    accum_out=acc_mm[:, i:i + 1])
                q2 = mp.tile([P, w], bf16, tag="q2")
                nc.vector.scalar_tensor_tensor(
                    q2[:], mx[:], 0.0, my[:], op0=alu.add, op1=alu.is_ge,
                    accum_out=acc_me[:, i:i + 1])

            # ---- final per-partition fold + store
            out_t = cp.tile([P, 4], f32)
            nc.vector.reduce_sum(out_t[:], acc_all[:], axis=X)
            nc.sync.dma_start(out_d[:], out_t[:])

    # Single activation table containing both Exp and Ln so the compiler
    # does not ping-pong ACT_TABLE_LOADs.
    import concourse.bacc as bacc_mod
    from concourse.hw_specs import get_activation_tables
    orig = get_activation_tables(nc.m.arch)
    combined = None
    for k, v in orig.items():
        if (mybir.ActivationFunctionType.Exp in v
                and mybir.ActivationFunctionType.Ln in v):
            combined = k
            break
    if combined is not None:
        patched = {k: (v if k == combined else set()) for k, v in orig.items()}
        saved = bacc_mod.get_activation_tables
        bacc_mod.get_activation_tables = lambda arch: patched
        try:
            nc.compile()
        finally:
            bacc_mod.get_activation_tables = saved
    else:
        nc.compile()
    return nc


def _get_nc():
    key = _CACHE["tiles_key"]
    if ("nc", key) not in _CACHE:
        _CACHE[("nc", key)] = _build_nc(*key)
    return _CACHE[("nc", key)]


# ------------------------------------------------------------------- host
def _host_prep(pred, target):
    """Shard, region-sort, permute slots, pad, tile. Returns in_maps and
    stores layout info in _CACHE."""
    pred = np.asarray(pred, dtype=np.float32)
    target = np.asarray(target).astype(np.int64)

    predp = pred.astype(ml_dtypes.bfloat16)
    colidx = _IDX_TAB[target]                      # [N, 10]
    packed = np.take_along_axis(predp, colidx, axis=1)
    bt = ((target >= 2) & (target <= 7))

    # per-core region rows
    core_a, core_b = [], []
    for c in range(N_CORES):
        sl = slice(c * ROWS_CORE, (c + 1) * ROWS_CORE)
        btc = bt[sl]
        pc = packed[sl]
        core_a.append(pc[~btc])
        core_b.append(pc[btc])

    wa = max((a.shape[0] + P - 1) // P for a in core_a)
    wb = max((b.shape[0] + P - 1) // P for b in core_b)
    tiles_a = _split_w(wa)
    tiles_b = _split_w(wb)
    _CACHE["tiles_key"] = (tuple(tiles_a), tuple(tiles_b))

    n_pad = 0
    in_maps = []
    for c in range(N_CORES):
        m = {}
        i = 0
        for rows_all, w_tot, tiles in ((core_a[c], wa, tiles_a),
                                       (core_b[c], wb, tiles_b)):
            n = rows_all.shape[0]
            pad = P * w_tot - n
            n_pad += pad
            if pad:
                rows_all = np.concatenate(
                    [rows_all, np.zeros((pad, C), ml_dtypes.bfloat16)], axis=0)
            # [P*w, 10] -> [P, w, 10] partition-major
            rows_all = rows_all.reshape(P, w_tot, C)
            off = 0
            for w in tiles:
                m[f"comb{i}"] = np.ascontiguousarray(
                    rows_all[:, off:off + w, :].reshape(P, w * C))
                off += w
                i += 1
        in_maps.append(m)
    _CACHE["n_pad"] = n_pad
    return in_maps


def _pad_ln_const():
    """Pad rows are all-zero: E = exp(0) = 1 exactly, bf16 tree sums to 10
    exactly, so the device adds ln(10) per pad row."""
    return float(np.log(10.0))


def kernel(pred, target):
    from concourse.bass_utils import run_bass_kernel_spmd

    in_maps = _host_prep(pred, target)
    nc = _get_nc()
    res = run_bass_kernel_spmd(nc, in_maps, core_ids=list(range(N_CORES)))

    sum_lg = 0.0
    sum_g = 0.0
    sum_mm = 0.0
    for c in range(N_CORES):
        o = res.results[c]["out"].astype(np.float64)
        sum_lg += o[:, 0].sum()
        sum_g += o[:, 1].sum()
        sum_mm += 0.5 * (o[:, 2].sum() + o[:, 3].sum())

    # pad rows have maxX == maxY == 0, so is_ge fires on every one of them
    sum_mm -= 0.5 * _CACHE["n_pad"]

    # pad rows: lnS = pad const, pred[t] slot = 0, no mismatch
    sum_lg -= _CACHE["n_pad"] * _pad_ln_const()
    ce = (sum_lg - sum_g) / N
    bce = 100.0 * sum_mm / N
    return np.float32(ce + bce)


# revision 13
# speedup vs baseline: 1.2807x; 1.2799x over previous
"""Trainium2 Bass kernel for nn_CustomLoss (CrossEntropy + binary-remap BCE).

loss = mean_i[ logsumexp(pred_i) - pred_i[t_i] ]
     + 100 * mean_i[ 1{ LUT[argmax(pred_i)] != LUT[t_i] } ]

with LUT = [0,0,1,1,1,1,1,1,0,0]  (LUT[j] = 1 iff 2 <= j <= 7).

Data-parallel over the batch across 8 NeuronCores.  The host re-packs rows
so the device needs neither a gather nor a sign trick:

  * rows are partitioned into region A (binary target bt=0, t in {0,1,8,9})
    and region B (bt=1, t in 2..7);
  * each row's 10 logits are permuted to  [X-group | pred[t] | Y-group-rest]
    where X = the class group NOT containing t and Y = the group containing
    t.  Then for every row, mismatch == (max X > max Y), and pred[t] sits at
    a fixed slot (6 in region A, 4 in region B);
  * tiles ship bf16 PLANE-MAJOR [P, 10, w] (slot-planes contiguous), so
    every DVE op below reads/writes packed 2-byte planes and hits the DVE
    2x fast mode.  DVE 2-port ops contend with GPSIMD for an SBUF read
    port, so GPSIMD is left completely idle.

Device per tile:
  ACT : E = exp(flat bf16), Ln(rowsum) with per-partition accumulate.
        Single Exp+Ln table (no reloads).
  DVE : plane add-tree for rowsum (9 adds/row, all 2x), plane max-trees
        (8 cmp/row, all 2x), d = maxX-maxY (2x), two tensor_scalar
        compares vs 0 with accum_out (4x) for the tie-corrected mismatch
        count, tensor_scalar accum (4x) for sum of pred[t].

Counting: bf16 rounding is monotone, so maxes of rounded values are the
rounded true maxes; ties get 0.5 credit via (is_gt + is_ge)/2.
"""

import numpy as np
import ml_dtypes

# ---------------------------------------------------------------- constants
N = 2_000_000
C = 10
N_CORES = 8
P = 128
ROWS_CORE = N // N_CORES      # 250,000
MID = (2, 3, 4, 5, 6, 7)      # classes with LUT == 1
OUTER = (0, 1, 8, 9)          # classes with LUT == 0

# slot permutation per target class: [X-group | t | Y-group minus t]
_IDX_TAB = np.empty((C, C), np.int64)
for _t in range(C):
    if _t in OUTER:   # region A: X = mid(6), Y = t + outer\{t}(3)
        _IDX_TAB[_t] = list(MID) + [_t] + [c for c in OUTER if c != _t]
    else:             # region B: X = outer(4), Y = t + mid\{t}(5)
        _IDX_TAB[_t] = list(OUTER) + [_t] + [c for c in MID if c != _t]

_CACHE = {}


def _split_w(w, first=140, body=348):
    """Tile widths: small first tile to prime the pipeline."""
    ws = []
    if w > first + 64:
        ws.append(first)
        w -= first
    while w > 0:
        c = min(body, w)
        if 0 < w - c < 64:
            c = w          # avoid a tiny trailing tile
        ws.append(c)
        w -= c
    return ws


# ------------------------------------------------------------- device build
def _build_nc(tiles_a, tiles_b):
    import concourse.bass as bass  # noqa: F401  (env setup)
    import concourse.tile as tile
    from concourse import bacc, mybir

    f32 = mybir.dt.float32
    bf16 = mybir.dt.bfloat16
    A = mybir.ActivationFunctionType
    alu = mybir.AluOpType

    tiles = [("a", w) for w in tiles_a] + [("b", w) for w in tiles_b]
    T = len(tiles)

    nc = bacc.Bacc("TRN2", target_bir_lowering=False, debug=False,
                   num_devices=N_CORES)
    comb_ds = [
        nc.dram_tensor(f"comb{i}", [P, C * w], bf16, kind="ExternalInput").ap()
        for i, (_, w) in enumerate(tiles)
    ]
    out_d = nc.dram_tensor("out", [P, 4], f32, kind="ExternalOutput").ap()

    with tile.TileContext(nc) as tc:
        with (
            tc.tile_pool(name="io", bufs=3) as io,
            tc.tile_pool(name="ep", bufs=3) as ep,
            tc.tile_pool(name="zp", bufs=3) as zp,
            tc.tile_pool(name="mp", bufs=3) as mp,
            tc.tile_pool(name="cp", bufs=1) as cp,
        ):
            acc_all = cp.tile([P, 4, T], f32)
            acc_lg = acc_all[:, 0, :]
            acc_g = acc_all[:, 1, :]
            acc_mm = acc_all[:, 2, :]
            acc_me = acc_all[:, 3, :]

            for i, (reg, w) in enumerate(tiles):
                ct = io.tile([P, C * w], bf16, tag="comb")
                nc.sync.dma_start(ct[:], comb_ds[i])
                cv = ct[:].rearrange("p (s w) -> p s w", s=C)

                # ---- CE: exp (flat), plane add tree, ln with accumulate
                et = ep.tile([P, C * w], bf16, tag="E")
                nc.scalar.activation(et[:], ct[:], A.Exp)
                ev = et[:].rearrange("p (s w) -> p s w", s=C)

                z5 = zp.tile([P, 5, w], bf16, tag="z5")
                nc.vector.tensor_tensor(z5[:], ev[:, 0:5, :], ev[:, 5:10, :],
                                        op=alu.add)
                z2 = zp.tile([P, 2, w], bf16, tag="z2")
                nc.vector.tensor_tensor(z2[:], z5[:, 0:2, :], z5[:, 2:4, :],
                                        op=alu.add)
                dd = zp.tile([P, w], bf16, tag="dd")
                nc.vector.tensor_tensor(dd[:], z2[:, 0, :], z2[:, 1, :],
                                        op=alu.add)
                s = zp.tile([P, w], bf16, tag="s")
                nc.vector.tensor_tensor(s[:], dd[:], z5[:, 4, :], op=alu.add)
                lg = zp.tile([P, w], f32, tag="lg")
                nc.scalar.activation(lg[:], s[:], A.Ln,
                                     accum_out=acc_lg[:, i:i + 1])

                # ---- sum of pred[t]: fixed plane per region
                u = 6 if reg == "a" else 4
                nc.vector.reduce_sum(acc_g[:, i:i + 1], cv[:, u, :],
                                     axis=mybir.AxisListType.X)

                # ---- BCE: plane max trees (all packed -> 2x)
                mx = mp.tile([P, w], bf16, tag="mx")
                my = mp.tile([P, w], bf16, tag="my")
                if reg == "a":
                    t1 = mp.tile([P, 2, w], bf16, tag="t1")
                    nc.vector.tensor_tensor(t1[:], cv[:, 0:2, :],
                                            cv[:, 2:4, :], op=alu.max)
                    t2 = mp.tile([P, 2, w], bf16, tag="t2")
                    nc.vector.tensor_tensor(t2[:], t1[:], cv[:, 4:6, :],
                                            op=alu.max)
                    nc.vector.tensor_tensor(mx[:], t2[:, 0, :], t2[:, 1, :],
                                            op=alu.max)
                    u1 = mp.tile([P, 2, w], bf16, tag="u1")
                    nc.vector.tensor_tensor(u1[:], cv[:, 6:8, :],
                                            cv[:, 8:10, :], op=alu.max)
                    nc.vector.tensor_tensor(my[:], u1[:, 0, :], u1[:, 1, :],
                                            op=alu.max)
                else:
                    t1 = mp.tile([P, 2, w], bf16, tag="t1")
                    nc.vector.tensor_tensor(t1[:], cv[:, 0:2, :],
                                            cv[:, 2:4, :], op=alu.max)
                    nc.vector.tensor_tensor(mx[:], t1[:, 0, :], t1[:, 1, :],
                                            op=alu.max)
                    u1 = mp.tile([P, 2, w], bf16, tag="u1")
                    nc.vector.tensor_tensor(u1[:], cv[:, 4:6, :],
                                            cv[:, 6:8, :], op=alu.max)
                    u2 = mp.tile([P, 2, w], bf16, tag="u2")
                    nc.vector.tensor_tensor(u2[:], u1[:], cv[:, 8:10, :],
                                            op=alu.max)
                    nc.vector.tensor_tensor(my[:], u2[:, 0, :], u2[:, 1, :],
                                            op=alu.max)

                # ---- tie-corrected count: (is_gt + is_ge)/2 via STT accum
                qg = mp.tile([P, w], bf16, tag="qg")
                nc.vector.scalar_tensor_tensor(
                    qg[:], mx[:], 0.0, my[:], op0=alu.add, op1=alu.is_gt,
                    accum_out=acc_mm[:, i:i + 1])
                qe = mp.tile([P, w], bf16, tag="qe")
                nc.vector.scalar_tensor_tensor(
                    qe[:], mx[:], 0.0, my[:], op0=alu.add, op1=alu.is_ge,
                    accum_out=acc_me[:, i:i + 1])

            # ---- final per-partition fold + store
            out_t = cp.tile([P, 4], f32)
            nc.vector.reduce_sum(out_t[:], acc_all[:], axis=mybir.AxisListType.X)
            nc.sync.dma_start(out_d[:], out_t[:])

    # Single activation table containing both Exp and Ln so the compiler
    # does not ping-pong ACT_TABLE_LOADs.
    import concourse.bacc as bacc_mod
    from concourse.hw_specs import get_activation_tables
    orig = get_activation_tables(nc.m.arch)
    combined = None
    for k, v in orig.items():
        if (mybir.ActivationFunctionType.Exp in v
                and mybir.ActivationFunctionType.Ln in v):
            combined = k
            break
    if combined is not None:
        patched = {k: (v if k == combined else set()) for k, v in orig.items()}
        saved = bacc_mod.get_activation_tables
        bacc_mod.get_activation_tables = lambda arch: patched
        try:
            nc.compile()
        finally:
            bacc_mod.get_activation_tables = saved
    else:
        nc.compile()
    return nc


def _get_nc():
    key = _CACHE["tiles_key"]
    if ("nc", key) not in _CACHE:
        _CACHE[("nc", key)] = _build_nc(*key)
    return _CACHE[("nc", key)]


# ------------------------------------------------------------------- host
def _host_prep(pred, target):
    """Shard, region-sort, permute slots, pad, tile (plane-major)."""
    pred = np.asarray(pred, dtype=np.float32)
    target = np.asarray(target).astype(np.int64)

    predb = pred.astype(ml_dtypes.bfloat16)
    colidx = _IDX_TAB[target]                      # [N, 10]
    packed = np.take_along_axis(predb, colidx, axis=1)
    bt = ((target >= 2) & (target <= 7))

    core_a, core_b = [], []
    for c in range(N_CORES):
        sl = slice(c * ROWS_CORE, (c + 1) * ROWS_CORE)
        btc = bt[sl]
        pc = packed[sl]
        core_a.append(pc[~btc])
        core_b.append(pc[btc])

    wa = max((a.shape[0] + P - 1) // P for a in core_a)
    wb = max((b.shape[0] + P - 1) // P for b in core_b)
    tiles_a = _split_w(wa)
    tiles_b = _split_w(wb)
    _CACHE["tiles_key"] = (tuple(tiles_a), tuple(tiles_b))

    n_pad = 0
    in_maps = []
    for c in range(N_CORES):
        m = {}
        i = 0
        for rows_all, w_tot, tiles in ((core_a[c], wa, tiles_a),
                                       (core_b[c], wb, tiles_b)):
            n = rows_all.shape[0]
            pad = P * w_tot - n
            n_pad += pad
            if pad:
                rows_all = np.concatenate(
                    [rows_all, np.zeros((pad, C), ml_dtypes.bfloat16)], axis=0)
            # [P*w, 10] -> [P, w, 10] partition-major -> [P, 10, w] planes
            pm = rows_all.reshape(P, w_tot, C).transpose(0, 2, 1)
            off = 0
            for w in tiles:
                m[f"comb{i}"] = np.ascontiguousarray(
                    pm[:, :, off:off + w].reshape(P, C * w))
                off += w
                i += 1
        in_maps.append(m)
    _CACHE["n_pad"] = n_pad
    return in_maps


def kernel(pred, target):
    from concourse.bass_utils import run_bass_kernel_spmd

    in_maps = _host_prep(pred, target)
    nc = _get_nc()
    res = run_bass_kernel_spmd(nc, in_maps, core_ids=list(range(N_CORES)))

    sum_lg = 0.0
    sum_g = 0.0
    sum_mm = 0.0
    for c in range(N_CORES):
        o = res.results[c]["out"].astype(np.float64)
        sum_lg += o[:, 0].sum()
        sum_g += o[:, 1].sum()
        sum_mm += 0.5 * (o[:, 2].sum() + o[:, 3].sum())

    # pad rows: all-zero slots -> E = 1 exactly, S = 10 exactly (bf16 tree),
    # lnS = ln 10; d = 0 so only is_ge fires on pads.
    n_pad = _CACHE["n_pad"]
    sum_lg -= n_pad * np.log(10.0)
    sum_mm -= 0.5 * n_pad
    ce = (sum_lg - sum_g) / N
    bce = 100.0 * sum_mm / N
    return np.float32(ce + bce)


# revision 15
# speedup vs baseline: 1.3599x; 1.0618x over previous
"""Trainium2 Bass kernel for nn_CustomLoss (CrossEntropy + binary-remap BCE).

loss = mean_i[ logsumexp(pred_i) - pred_i[t_i] ]
     + 100 * mean_i[ 1{ LUT[argmax(pred_i)] != LUT[t_i] } ]

with LUT = [0,0,1,1,1,1,1,1,0,0]  (LUT[j] = 1 iff 2 <= j <= 7).

Data-parallel over the batch across 8 NeuronCores.  The host re-packs rows
so the device needs neither a gather nor a sign trick:

  * rows are partitioned into region A (binary target bt=0, t in {0,1,8,9})
    and region B (bt=1, t in 2..7);
  * each row's 10 logits are permuted to  [X-group | pred[t] | Y-group-rest]
    where X = the class group NOT containing t and Y = the group containing
    t.  Then for every row, mismatch == (max X > max Y), and pred[t] sits at
    a fixed slot (6 in region A, 4 in region B);
  * tiles ship bf16 PLANE-MAJOR [P, 10, w] (slot-planes contiguous), so
    every DVE op below reads/writes packed 2-byte planes and hits the DVE
    2x fast mode.  DVE 2-port ops contend with GPSIMD for an SBUF read
    port, so GPSIMD is left completely idle.

Device per tile:
  ACT : E = exp(flat bf16), Ln(rowsum) with per-partition accumulate.
        Single Exp+Ln table (no reloads).
  DVE : plane add-tree for rowsum (9 adds/row, all 2x), plane max-trees
        (8 cmp/row, all 2x), d = maxX-maxY (2x), two tensor_scalar
        compares vs 0 with accum_out (4x) for the tie-corrected mismatch
        count, tensor_scalar accum (4x) for sum of pred[t].

Counting: bf16 rounding is monotone, so maxes of rounded values are the
rounded true maxes; ties get 0.5 credit via (is_gt + is_ge)/2.
"""

import numpy as np
import ml_dtypes

# ---------------------------------------------------------------- constants
N = 2_000_000
C = 10
N_CORES = 8
P = 128
ROWS_CORE = N // N_CORES      # 250,000
MID = (2, 3, 4, 5, 6, 7)      # classes with LUT == 1
OUTER = (0, 1, 8, 9)          # classes with LUT == 0

# slot permutation per target class: [X-group | t | Y-group minus t]
_IDX_TAB = np.empty((C, C), np.int64)
for _t in range(C):
    if _t in OUTER:   # region A: X = mid(6), Y = t + outer\{t}(3)
        _IDX_TAB[_t] = list(MID) + [_t] + [c for c in OUTER if c != _t]
    else:             # region B: X = outer(4), Y = t + mid\{t}(5)
        _IDX_TAB[_t] = list(OUTER) + [_t] + [c for c in MID if c != _t]

_CACHE = {}


def _split_w(w, first=250, body=520):
    """Tile widths: small first tile to prime the pipeline."""
    ws = []
    if w > first + 64:
        ws.append(first)
        w -= first
    while w > 0:
        c = min(body, w)
        if 0 < w - c < 64:
            c = w          # avoid a tiny trailing tile
        ws.append(c)
        w -= c
    return ws


# ------------------------------------------------------------- device build
def _build_nc(tiles_a, tiles_b):
    import concourse.bass as bass  # noqa: F401  (env setup)
    import concourse.tile as tile
    from concourse import bacc, mybir

    f32 = mybir.dt.float32
    bf16 = mybir.dt.bfloat16
    A = mybir.ActivationFunctionType
    alu = mybir.AluOpType

    tiles = [("a", w) for w in tiles_a] + [("b", w) for w in tiles_b]
    T = len(tiles)

    nc = bacc.Bacc("TRN2", target_bir_lowering=False, debug=False,
                   num_devices=N_CORES)
    comb_ds = [
        nc.dram_tensor(f"comb{i}", [P, C * w], bf16, kind="ExternalInput").ap()
        for i, (_, w) in enumerate(tiles)
    ]
    out_d = nc.dram_tensor("out", [P, 4], f32, kind="ExternalOutput").ap()

    with tile.TileContext(nc) as tc:
        with (
            tc.tile_pool(name="io", bufs=3) as io,
            tc.tile_pool(name="ep", bufs=3) as ep,
            tc.tile_pool(name="zp", bufs=3) as zp,
            tc.tile_pool(name="mp", bufs=3) as mp,
            tc.tile_pool(name="cp", bufs=1) as cp,
        ):
            acc_all = cp.tile([P, 4, T], f32)
            acc_lg = acc_all[:, 0, :]
            acc_g = acc_all[:, 1, :]
            acc_mm = acc_all[:, 2, :]
            acc_me = acc_all[:, 3, :]

            for i, (reg, w) in enumerate(tiles):
                ct = io.tile([P, C * w], bf16, tag="comb")
                nc.sync.dma_start(ct[:], comb_ds[i])
                cv = ct[:].rearrange("p (s w) -> p s w", s=C)

                # ---- CE: exp (flat), plane add tree, ln with accumulate
                et = ep.tile([P, C * w], bf16, tag="E")
                nc.scalar.activation(et[:], ct[:], A.Exp)
                ev = et[:].rearrange("p (s w) -> p s w", s=C)

                z5 = zp.tile([P, 5, w], bf16, tag="z5")
                nc.vector.tensor_tensor(z5[:], ev[:, 0:5, :], ev[:, 5:10, :],
                                        op=alu.add)
                z2 = zp.tile([P, 2, w], bf16, tag="z2")
                nc.vector.tensor_tensor(z2[:], z5[:, 0:2, :], z5[:, 2:4, :],
                                        op=alu.add)
                dd = zp.tile([P, w], bf16, tag="dd")
                nc.vector.tensor_tensor(dd[:], z2[:, 0, :], z2[:, 1, :],
                                        op=alu.add)
                s = zp.tile([P, w], bf16, tag="s")
                nc.vector.tensor_tensor(s[:], dd[:], z5[:, 4, :], op=alu.add)
                lg = zp.tile([P, w], f32, tag="lg")
                nc.scalar.activation(lg[:], s[:], A.Ln,
                                     accum_out=acc_lg[:, i:i + 1])

                # ---- sum of pred[t]: fixed plane per region
                u = 6 if reg == "a" else 4
                nc.vector.reduce_sum(acc_g[:, i:i + 1], cv[:, u, :],
                                     axis=mybir.AxisListType.X)

                # ---- BCE: merged plane max trees (3 ops, all packed -> 2x)
                # cvg[p, g, s, w] groups the 10 planes into 5 pairs.
                cvg = ct[:].rearrange("p (g s w) -> p g s w", s=2, w=w)
                tu = mp.tile([P, 2, 2, w], bf16, tag="tu")
                if reg == "a":
                    # X = planes 0..5, Y = 6..9
                    # tu[0] = max(pair g0, pair g1) covers planes 0..3
                    # tu[1] = max(pair g3, pair g4) covers planes 6..9 (=Y)
                    nc.vector.tensor_tensor(tu[:], cvg[:, 0:4:3, :, :],
                                            cvg[:, 1:5:3, :, :], op=alu.max)
                    # fold planes (4,5) into the X half
                    nc.vector.tensor_tensor(tu[:, 0, :, :], tu[:, 0, :, :],
                                            cvg[:, 2, :, :], op=alu.max)
                else:
                    # X = planes 0..3, Y = 4..9
                    # tu[0] = max(pair g0, pair g1) covers planes 0..3 (=X)
                    # tu[1] = max(pair g2, pair g3) covers planes 4..7
                    nc.vector.tensor_tensor(tu[:], cvg[:, 0:3:2, :, :],
                                            cvg[:, 1:4:2, :, :], op=alu.max)
                    # fold planes (8,9) into the Y half
                    nc.vector.tensor_tensor(tu[:, 1, :, :], tu[:, 1, :, :],
                                            cvg[:, 4, :, :], op=alu.max)
                mxy = mp.tile([P, 2, w], bf16, tag="mxy")
                nc.vector.tensor_tensor(mxy[:], tu[:, :, 0, :],
                                        tu[:, :, 1, :], op=alu.max)
                mx = mxy[:, 0, :]
                my = mxy[:, 1, :]

                # ---- tie-corrected count: (is_gt + is_ge)/2 via STT accum
                qg = mp.tile([P, w], bf16, tag="qg")
                nc.vector.scalar_tensor_tensor(
                    qg[:], mx, 0.0, my, op0=alu.add, op1=alu.is_gt,
                    accum_out=acc_mm[:, i:i + 1])
                qe = mp.tile([P, w], bf16, tag="qe")
                nc.vector.scalar_tensor_tensor(
                    qe[:], mx, 0.0, my, op0=alu.add, op1=alu.is_ge,
                    accum_out=acc_me[:, i:i + 1])

            # ---- final per-partition fold + store
            out_t = cp.tile([P, 4], f32)
            nc.vector.reduce_sum(out_t[:], acc_all[:], axis=mybir.AxisListType.X)
            nc.sync.dma_start(out_d[:], out_t[:])

    # Single activation table containing both Exp and Ln so the compiler
    # does not ping-pong ACT_TABLE_LOADs.
    import concourse.bacc as bacc_mod
    from concourse.hw_specs import get_activation_tables
    orig = get_activation_tables(nc.m.arch)
    combined = None
    for k, v in orig.items():
        if (mybir.ActivationFunctionType.Exp in v
                and mybir.ActivationFunctionType.Ln in v):
            combined = k
            break
    if combined is not None:
        patched = {k: (v if k == combined else set()) for k, v in orig.items()}
        saved = bacc_mod.get_activation_tables
        bacc_mod.get_activation_tables = lambda arch: patched
        try:
            nc.compile()
        finally:
            bacc_mod.get_activation_tables = saved
    else:
        nc.compile()
    return nc


def _get_nc():
    key = _CACHE["tiles_key"]
    if ("nc", key) not in _CACHE:
        _CACHE[("nc", key)] = _build_nc(*key)
    return _CACHE[("nc", key)]


# ------------------------------------------------------------------- host
def _host_prep(pred, target):
    """Shard, region-sort, permute slots, pad, tile (plane-major)."""
    pred = np.asarray(pred, dtype=np.float32)
    target = np.asarray(target).astype(np.int64)

    predb = pred.astype(ml_dtypes.bfloat16)
    colidx = _IDX_TAB[target]                      # [N, 10]
    packed = np.take_along_axis(predb, colidx, axis=1)
    bt = ((target >= 2) & (target <= 7))

    core_a, core_b = [], []
    for c in range(N_CORES):
        sl = slice(c * ROWS_CORE, (c + 1) * ROWS_CORE)
        btc = bt[sl]
        pc = packed[sl]
        core_a.append(pc[~btc])
        core_b.append(pc[btc])

    wa = max((a.shape[0] + P - 1) // P for a in core_a)
    wb = max((b.shape[0] + P - 1) // P for b in core_b)
    tiles_a = _split_w(wa)
    tiles_b = _split_w(wb)
    _CACHE["tiles_key"] = (tuple(tiles_a), tuple(tiles_b))

    n_pad = 0
    in_maps = []
    for c in range(N_CORES):
        m = {}
        i = 0
        for rows_all, w_tot, tiles in ((core_a[c], wa, tiles_a),
                                       (core_b[c], wb, tiles_b)):
            n = rows_all.shape[0]
            pad = P * w_tot - n
            n_pad += pad
            if pad:
                rows_all = np.concatenate(
                    [rows_all, np.zeros((pad, C), ml_dtypes.bfloat16)], axis=0)
            # [P*w, 10] -> [P, w, 10] partition-major -> [P, 10, w] planes
            pm = rows_all.reshape(P, w_tot, C).transpose(0, 2, 1)
            off = 0
            for w in tiles:
                m[f"comb{i}"] = np.ascontiguousarray(
                    pm[:, :, off:off + w].reshape(P, C * w))
                off += w
                i += 1
        in_maps.append(m)
    _CACHE["n_pad"] = n_pad
    return in_maps


def kernel(pred, target):
    from concourse.bass_utils import run_bass_kernel_spmd

    in_maps = _host_prep(pred, target)
    nc = _get_nc()
    res = run_bass_kernel_spmd(nc, in_maps, core_ids=list(range(N_CORES)))

    sum_lg = 0.0
    sum_g = 0.0
    sum_mm = 0.0
    for c in range(N_CORES):
        o = res.results[c]["out"].astype(np.float64)
        sum_lg += o[:, 0].sum()
        sum_g += o[:, 1].sum()
        sum_mm += 0.5 * (o[:, 2].sum() + o[:, 3].sum())

    # pad rows: all-zero slots -> E = 1 exactly, S = 10 exactly (bf16 tree),
    # lnS = ln 10; d = 0 so only is_ge fires on pads.
    n_pad = _CACHE["n_pad"]
    sum_lg -= n_pad * np.log(10.0)
    sum_mm -= 0.5 * n_pad
    ce = (sum_lg - sum_g) / N
    bce = 100.0 * sum_mm / N
    return np.float32(ce + bce)


# revision 20
# speedup vs baseline: 1.4792x; 1.0877x over previous
"""Trainium2 Bass kernel for nn_CustomLoss (CrossEntropy + binary-remap BCE).

loss = mean_i[ logsumexp(pred_i) - pred_i[t_i] ]
     + 100 * mean_i[ 1{ LUT[argmax(pred_i)] != LUT[t_i] } ]

with LUT = [0,0,1,1,1,1,1,1,0,0]  (LUT[j] = 1 iff 2 <= j <= 7).

Data-parallel over the batch across 8 NeuronCores.  The host re-packs rows
so the device needs neither a gather nor a sign trick:

  * rows are partitioned into region A (binary target bt=0, t in {0,1,8,9})
    and region B (bt=1, t in 2..7);
  * each row's 10 logits are permuted to  [X-group | pred[t] | Y-group-rest]
    where X = the class group NOT containing t and Y = the group containing
    t.  Then for every row, mismatch == (max X > max Y), and pred[t] sits at
    a fixed slot (6 in region A, 4 in region B);
  * tiles ship bf16 PLANE-MAJOR [P, 10, w] (slot-planes contiguous), so
    every DVE op below reads/writes packed 2-byte planes and hits the DVE
    2x fast mode.  DVE 2-port ops contend with GPSIMD for an SBUF read
    port, so GPSIMD is left completely idle.

Device per tile:
  ACT : E = exp(flat bf16), Ln(rowsum) with per-partition accumulate.
        Single Exp+Ln table (no reloads).
  DVE : plane add-tree for rowsum (9 adds/row, all 2x), plane max-trees
        (8 cmp/row, all 2x), d = maxX-maxY (2x), two tensor_scalar
        compares vs 0 with accum_out (4x) for the tie-corrected mismatch
        count, tensor_scalar accum (4x) for sum of pred[t].

Counting: bf16 rounding is monotone, so maxes of rounded values are the
rounded true maxes; ties get 0.5 credit via (is_gt + is_ge)/2.
"""

import numpy as np
import ml_dtypes

# ---------------------------------------------------------------- constants
N = 2_000_000
C = 10
N_CORES = 8
P = 128
ROWS_CORE = N // N_CORES      # 250,000
MID = (2, 3, 4, 5, 6, 7)      # classes with LUT == 1
OUTER = (0, 1, 8, 9)          # classes with LUT == 0

# slot permutation per target class: [X-group | t | Y-group minus t]
_IDX_TAB = np.empty((C, C), np.int64)
for _t in range(C):
    if _t in OUTER:   # region A: X = mid(6), Y = t + outer\{t}(3)
        _IDX_TAB[_t] = list(MID) + [_t] + [c for c in OUTER if c != _t]
    else:             # region B: X = outer(4), Y = t + mid\{t}(5)
        _IDX_TAB[_t] = list(OUTER) + [_t] + [c for c in MID if c != _t]

_CACHE = {}


def _split_w(w, first=270, body=512):
    """Tile widths: small first tile to prime the pipeline."""
    ws = []
    if w > first + 64:
        ws.append(first)
        w -= first
    while w > 0:
        c = min(body, w)
        if 0 < w - c < 64:
            c = (w + 1) // 2   # split evenly instead of a tiny trailing tile
        ws.append(c)
        w -= c
    return ws


# ------------------------------------------------------------- device build
def _build_nc(tiles_a, tiles_b):
    import concourse.bass as bass  # noqa: F401  (env setup)
    import concourse.tile as tile
    from concourse import bacc, mybir

    f32 = mybir.dt.float32
    bf16 = mybir.dt.bfloat16
    A = mybir.ActivationFunctionType
    alu = mybir.AluOpType

    tiles = [("a", w) for w in tiles_a] + [("b", w) for w in tiles_b]
    T = len(tiles)

    nc = bacc.Bacc("TRN2", target_bir_lowering=False, debug=False,
                   num_devices=N_CORES)
    comb_ds = [
        nc.dram_tensor(f"comb{i}", [P, C * w], bf16, kind="ExternalInput").ap()
        for i, (_, w) in enumerate(tiles)
    ]
    out_d = nc.dram_tensor("out", [P, 4], f32, kind="ExternalOutput").ap()

    from concourse.masks import make_identity

    with tile.TileContext(nc) as tc:
        with (
            tc.tile_pool(name="io", bufs=3) as io,
            tc.tile_pool(name="ep", bufs=3) as ep,
            tc.psum_pool(name="ps", bufs=3) as ps,
            tc.tile_pool(name="mp", bufs=3) as mp,
            tc.tile_pool(name="cp", bufs=1) as cp,
        ):
            ident = cp.tile([P, P], bf16)
            make_identity(nc, ident[:])
            acc_all = cp.tile([P, 4, T], f32)
            acc_lg = acc_all[:, 0, :]
            acc_g = acc_all[:, 1, :]
            acc_mm = acc_all[:, 2, :]
            acc_me = acc_all[:, 3, :]

            for i, (reg, w) in enumerate(tiles):
                ct = io.tile([P, C * w], bf16, tag="comb")
                nc.sync.dma_start(ct[:], comb_ds[i])
                cv = ct[:].rearrange("p (s w) -> p s w", s=C)

                # ---- CE: exp (flat), PE identity-matmul row sums into PSUM
                # (exact f32 accumulation), in-place Ln with accumulate
                et = ep.tile([P, C * w], bf16, tag="E")
                nc.scalar.activation(et[:], ct[:], A.Exp)
                ev = et[:].rearrange("p (s w) -> p s w", s=C)

                # full-bank psum tile keeps pool offsets bank-aligned
                pt = ps.tile([P, 512], f32, tag="S")
                for sl in range(C):
                    nc.tensor.matmul(pt[:, :w], ident[:], ev[:, sl, :],
                                     start=(sl == 0), stop=(sl == C - 1))
                nc.scalar.activation(pt[:, :w], pt[:, :w], A.Ln,
                                     accum_out=acc_lg[:, i:i + 1])

                # ---- sum of pred[t]: fixed plane per region
                u = 6 if reg == "a" else 4
                nc.vector.reduce_sum(acc_g[:, i:i + 1], cv[:, u, :],
                                     axis=mybir.AxisListType.X)

                # ---- BCE: merged plane max trees (3 ops, all packed -> 2x)
                # cvg[p, g, s, w] groups the 10 planes into 5 pairs.
                cvg = ct[:].rearrange("p (g s w) -> p g s w", s=2, w=w)
                tu = mp.tile([P, 2, 2, w], bf16, tag="tu")
                if reg == "a":
                    # X = planes 0..5, Y = 6..9
                    # tu[0] = max(pair g0, pair g1) covers planes 0..3
                    # tu[1] = max(pair g3, pair g4) covers planes 6..9 (=Y)
                    nc.vector.tensor_tensor(tu[:], cvg[:, 0:4:3, :, :],
                                            cvg[:, 1:5:3, :, :], op=alu.max)
                    # fold planes (4,5) into the X half
                    nc.vector.tensor_tensor(tu[:, 0, :, :], tu[:, 0, :, :],
                                            cvg[:, 2, :, :], op=alu.max)
                else:
                    # X = planes 0..3, Y = 4..9
                    # tu[0] = max(pair g0, pair g1) covers planes 0..3 (=X)
                    # tu[1] = max(pair g2, pair g3) covers planes 4..7
                    nc.vector.tensor_tensor(tu[:], cvg[:, 0:3:2, :, :],
                                            cvg[:, 1:4:2, :, :], op=alu.max)
                    # fold planes (8,9) into the Y half
                    nc.vector.tensor_tensor(tu[:, 1, :, :], tu[:, 1, :, :],
                                            cvg[:, 4, :, :], op=alu.max)
                mxy = mp.tile([P, 2, w], bf16, tag="mxy")
                nc.vector.tensor_tensor(mxy[:], tu[:, :, 0, :],
                                        tu[:, :, 1, :], op=alu.max)
                mx = mxy[:, 0, :]
                my = mxy[:, 1, :]

                # ---- tie-corrected count: (is_gt + is_ge)/2 via STT accum
                qg = mp.tile([P, w], bf16, tag="qg")
                nc.vector.scalar_tensor_tensor(
                    qg[:], mx, 0.0, my, op0=alu.add, op1=alu.is_gt,
                    accum_out=acc_mm[:, i:i + 1])
                qe = mp.tile([P, w], bf16, tag="qe")
                nc.vector.scalar_tensor_tensor(
                    qe[:], mx, 0.0, my, op0=alu.add, op1=alu.is_ge,
                    accum_out=acc_me[:, i:i + 1])

            # ---- final per-partition fold + store
            out_t = cp.tile([P, 4], f32)
            nc.vector.reduce_sum(out_t[:], acc_all[:], axis=mybir.AxisListType.X)
            nc.sync.dma_start(out_d[:], out_t[:])

    # Single activation table containing both Exp and Ln so the compiler
    # does not ping-pong ACT_TABLE_LOADs.
    import concourse.bacc as bacc_mod
    from concourse.hw_specs import get_activation_tables
    orig = get_activation_tables(nc.m.arch)
    combined = None
    for k, v in orig.items():
        if (mybir.ActivationFunctionType.Exp in v
                and mybir.ActivationFunctionType.Ln in v):
            combined = k
            break
    if combined is not None:
        patched = {k: (v if k == combined else set()) for k, v in orig.items()}
        saved = bacc_mod.get_activation_tables
        bacc_mod.get_activation_tables = lambda arch: patched
        try:
            nc.compile()
        finally:
            bacc_mod.get_activation_tables = saved
    else:
        nc.compile()
    return nc


def _get_nc():
    key = _CACHE["tiles_key"]
    if ("nc", key) not in _CACHE:
        _CACHE[("nc", key)] = _build_nc(*key)
    return _CACHE[("nc", key)]


# ------------------------------------------------------------------- host
def _host_prep(pred, target):
    """Shard, region-sort, permute slots, pad, tile (plane-major)."""
    pred = np.asarray(pred, dtype=np.float32)
    target = np.asarray(target).astype(np.int64)

    predb = pred.astype(ml_dtypes.bfloat16)
    colidx = _IDX_TAB[target]                      # [N, 10]
    packed = np.take_along_axis(predb, colidx, axis=1)
    bt = ((target >= 2) & (target <= 7))

    core_a, core_b = [], []
    for c in range(N_CORES):
        sl = slice(c * ROWS_CORE, (c + 1) * ROWS_CORE)
        btc = bt[sl]
        pc = packed[sl]
        core_a.append(pc[~btc])
        core_b.append(pc[btc])

    wa = max((a.shape[0] + P - 1) // P for a in core_a)
    wb = max((b.shape[0] + P - 1) // P for b in core_b)
    tiles_a = _split_w(wa)
    tiles_b = _split_w(wb)
    _CACHE["tiles_key"] = (tuple(tiles_a), tuple(tiles_b))

    n_pad = 0
    in_maps = []
    for c in range(N_CORES):
        m = {}
        i = 0
        for rows_all, w_tot, tiles in ((core_a[c], wa, tiles_a),
                                       (core_b[c], wb, tiles_b)):
            n = rows_all.shape[0]
            pad = P * w_tot - n
            n_pad += pad
            if pad:
                rows_all = np.concatenate(
                    [rows_all, np.zeros((pad, C), ml_dtypes.bfloat16)], axis=0)
            # [P*w, 10] -> [P, w, 10] partition-major -> [P, 10, w] planes
            pm = rows_all.reshape(P, w_tot, C).transpose(0, 2, 1)
            off = 0
            for w in tiles:
                m[f"comb{i}"] = np.ascontiguousarray(
                    pm[:, :, off:off + w].reshape(P, C * w))
                off += w
                i += 1
        in_maps.append(m)
    _CACHE["n_pad"] = n_pad
    return in_maps


def kernel(pred, target):
    from concourse.bass_utils import run_bass_kernel_spmd

    in_maps = _host_prep(pred, target)
    nc = _get_nc()
    res = run_bass_kernel_spmd(nc, in_maps, core_ids=list(range(N_CORES)))

    sum_lg = 0.0
    sum_g = 0.0
    sum_mm = 0.0
    for c in range(N_CORES):
        o = res.results[c]["out"].astype(np.float64)
        sum_lg += o[:, 0].sum()
        sum_g += o[:, 1].sum()
        sum_mm += 0.5 * (o[:, 2].sum() + o[:, 3].sum())

    # pad rows: all-zero slots -> E = 1 exactly, S = 10 exactly (bf16 tree),
    # lnS = ln 10; d = 0 so only is_ge fires on pads.
    n_pad = _CACHE["n_pad"]
    sum_lg -= n_pad * np.log(10.0)
    sum_mm -= 0.5 * n_pad
    ce = (sum_lg - sum_g) / N
    bce = 100.0 * sum_mm / N
    return np.float32(ce + bce)
